# revision 9
# baseline (speedup 1.0000x reference)
"""ClusterGCN + 2x GAT message-passing kernel for 8 Trainium2 NeuronCores.

Strategy (dst-sharded, one SPMD program):
  - Nodes are permuted into 784 tiles of 128 slots, load-balanced so every
    tile has (nearly) the same number of incoming edges (self-loops added).
    Cores own 98 consecutive tiles each.
  - Per GAT layer, each core fetches the z-rows of its incoming messages
    from a replicated, 256B-pitched node table via batched SWDGE dma_gather
    (InstDMAGatherAnt): one call per (7-tile batch, 25088-row chunk) --
    int16 indices force the 4-way chunk split; the ~1us Q7 fixed cost per
    call amortizes over ~4480 descriptors at 0.34ns each. Payloads are
    narrower than the row pitch (136B / 72B of the 256B row).
  - Messages are reduced per dst tile with 0/1 selection-matrix matmuls
    accumulated in PSUM; layer transforms run in feature-major space.
  - Per-edge s_dst comes from a second 4-byte-payload dma_gather out of a
    pitched f32 s-table (exact, no hi/lo tricks); per-edge s_src rides in
    the gathered z-row as a bf16 hi/lo pair.
  - exp(leaky_relu(l)) = max(exp(l), exp(0.2*l)): both exps on the Scalar
    engine (scale fused), freeing the Vector engine.
  - PSUM evacuation / scaling copies run on the Scalar engine.
  - Between layers the per-core z-tables are AllGathered so every core can
    gather arbitrary source rows; s tables stay core-local.
"""

import os
import sys

sys.path.insert(0, "/opt/trn_rl_repo")
os.environ.setdefault("NEURON_RT_RESET_CORES", "1")

import numpy as np

import concourse.bacc as bacc
import concourse.bass as bass
import concourse.mybir as mybir
import concourse.tile as tile
from concourse.bass_utils import run_bass_kernel_spmd

# ---- problem constants (hardcoded per contest rules) ----
N = 100000
E = 1600000
FIN = 64
HID = 64
FOUT = 32
NEG = 0.2

P = 128
NCORES = 8
TILES_PER_CORE = 98
T_ALL = NCORES * TILES_PER_CORE  # 784
NPC = TILES_PER_CORE * P  # 12544 nodes per core
NP_ALL = T_ALL * P  # 100352 padded node count

FW1 = 68  # z1 row payload: z(64) | 1.0 | s_hi | s_lo | pad
FW2 = 36  # z2 row payload: z(32) | 1.0 | s_hi | s_lo | pad
FWP = 128  # z row pitch in bf16 elements (256B, dma_gather requirement)
SDP = 64  # s table row pitch in f32 elements (256B)

NCHUNK = 4
CH = NP_ALL // NCHUNK  # 25088 rows per chunk (< 32768 for int16 idx)
KC = 5  # gather columns per (tile, chunk): fits max 593 messages
KU2 = 1 + NCHUNK * KC  # 21 message columns per tile in GAT layers
BATCH = 7  # dst tiles per gather batch
NBATCH = TILES_PER_CORE // BATCH  # 14
NIZ = BATCH * KC * P  # 4480 z-idxs per (batch, chunk)
NIS = BATCH * KU2 * P  # 18816 s-idxs per batch
GI = NCHUNK * (NIZ // 16) + NIS // 16  # 2296 idx cols per batch

F32 = mybir.dt.float32
BF16 = mybir.dt.bfloat16
I16 = mybir.dt.int16
AF = mybir.ActivationFunctionType
OP = mybir.AluOpType

_cache = {}
last_result = None


def _bf16(a):
    import ml_dtypes

    return np.asarray(a, dtype=ml_dtypes.bfloat16)


# ----------------------------------------------------------------------------
# host-side preprocessing
# ----------------------------------------------------------------------------
def _balance_tiles(deg):
    """Assign each of NP_ALL nodes to one of T_ALL tiles (128 slots each) so
    that per-tile total in-degree is near-uniform. Returns perm arrays."""
    import heapq

    order = np.argsort(-deg, kind="stable")
    heap = [(0, t) for t in range(T_ALL)]
    heapq.heapify(heap)
    counts = np.zeros(T_ALL, np.int64)
    loads = np.zeros(T_ALL, np.int64)
    tile_of = np.empty(NP_ALL, np.int32)
    slot_of = np.empty(NP_ALL, np.int32)
    for n in order:
        while True:
            load, t = heapq.heappop(heap)
            if counts[t] < P:
                break
        tile_of[n] = t
        slot_of[n] = counts[t]
        counts[t] += 1
        loads[t] += deg[n]
        if counts[t] < P:
            heapq.heappush(heap, (loads[t], t))
    return tile_of, slot_of, int(loads.max())


def _wrap16(flat):
    """dma_gather idx layout: flat i -> [i%16, i//16], replicated 8x to 128
    partitions."""
    n = len(flat)
    assert n % 16 == 0
    w = np.zeros((16, n // 16), np.int16)
    w[np.arange(n) % 16, np.arange(n) // 16] = flat
    return np.tile(w, (8, 1))


def _preprocess(x, edge_index):
    src = np.asarray(edge_index[0], np.int64)
    dst = np.asarray(edge_index[1], np.int64)
    loops = np.arange(NP_ALL, dtype=np.int64)
    deg = np.bincount(np.concatenate([dst, loops]), minlength=NP_ALL)

    tile_of, slot_of, max_load = _balance_tiles(deg)
    ku = (max_load + P - 1) // P
    gid = tile_of.astype(np.int64) * P + slot_of  # node -> permuted row

    # ---------------- layer-1 (17-col, edge-order xe) structures ------------
    src_all = np.concatenate([src, loops])
    dst_all = np.concatenate([dst, loops])
    m_src = gid[src_all]
    m_tile = tile_of[dst_all].astype(np.int64)
    m_slot = slot_of[dst_all].astype(np.int64)
    is_loop_m = np.concatenate([np.zeros(len(src), bool), np.ones(NP_ALL, bool)])
    order = np.lexsort((~is_loop_m, m_tile))  # loops first within each tile
    m_src1, m_tile1, m_slot1, is_loop1 = (
        m_src[order], m_tile[order], m_slot[order], is_loop_m[order],
    )
    tile_counts = np.bincount(m_tile1, minlength=T_ALL)
    tile_starts = np.concatenate([[0], np.cumsum(tile_counts)[:-1]])
    pos = np.arange(len(m_src1)) - tile_starts[m_tile1]
    mp1 = np.where(is_loop1, m_slot1, pos % P)
    mc1 = np.where(is_loop1, 0, pos // P)
    cols1 = TILES_PER_CORE * ku
    midx1 = np.zeros((NCORES, P, cols1), np.int32)
    mloc1 = np.full((NCORES, P, cols1), -1.0, np.float32)
    core1 = m_tile1 // TILES_PER_CORE
    tl1 = m_tile1 % TILES_PER_CORE
    col1 = tl1 * ku + mc1
    midx1[core1, mp1, col1] = m_src1
    mloc1[core1, mp1, col1] = m_slot1

    # ---------------- GAT-layer (21-col, chunked dma_gather) structures -----
    # non-self messages grouped by (dst tile, src chunk); rank within group
    # gives (col j, slot p); self-loops sit slot-aligned in col 0.
    e_src = gid[src]
    e_tile = tile_of[dst].astype(np.int64)
    e_slot = slot_of[dst].astype(np.int64)
    e_chunk = e_src // CH
    ordg = np.lexsort((e_chunk, e_tile))
    g_src, g_tile, g_slot, g_chunk = (
        e_src[ordg], e_tile[ordg], e_slot[ordg], e_chunk[ordg],
    )
    grp = g_tile * NCHUNK + g_chunk
    grp_counts = np.bincount(grp, minlength=T_ALL * NCHUNK)
    assert grp_counts.max() <= KC * P, grp_counts.max()
    grp_starts = np.concatenate([[0], np.cumsum(grp_counts)[:-1]])
    rank = np.arange(len(g_src)) - grp_starts[grp]
    g_j = rank // P
    g_p = rank % P
    g_col = 1 + g_chunk * KC + g_j  # message column 1..20
    g_sidx = g_slot * TILES_PER_CORE + (g_tile % TILES_PER_CORE)

    cols2 = TILES_PER_CORE * KU2
    mloc2 = np.full((NCORES, P, cols2), -1.0, np.float32)
    mdst2 = np.zeros((NCORES, P, cols2), np.int64)
    zidx2 = np.zeros((NCORES, P, cols2), np.int64)  # chunk-local src row
    coreg = g_tile // TILES_PER_CORE
    tlg = g_tile % TILES_PER_CORE
    colg = tlg * KU2 + g_col
    mloc2[coreg, g_p, colg] = g_slot
    mdst2[coreg, g_p, colg] = g_sidx
    zidx2[coreg, g_p, colg] = g_src - g_chunk * CH
    # self col 0 (slot-aligned)
    l_tile = tile_of[loops].astype(np.int64)
    l_slot = slot_of[loops].astype(np.int64)
    l_core = l_tile // TILES_PER_CORE
    l_tl = l_tile % TILES_PER_CORE
    l_col = l_tl * KU2
    mloc2[l_core, l_slot, l_col] = l_slot
    mdst2[l_core, l_slot, l_col] = l_slot * TILES_PER_CORE + l_tl

    # per-batch idx arrays (shared by both GAT layers)
    gidx = np.zeros((NCORES, NBATCH, P, GI), np.int16)
    zi = zidx2.reshape(NCORES, P, TILES_PER_CORE, KU2)
    si = mdst2.reshape(NCORES, P, TILES_PER_CORE, KU2)
    for c in range(NCORES):
        for bi in range(NBATCH):
            ts = slice(bi * BATCH, (bi + 1) * BATCH)
            parts = []
            for ch in range(NCHUNK):
                # flat i = (b*KC + j)*128 + p
                blk = zi[c, :, ts, 1 + ch * KC : 1 + (ch + 1) * KC]  # [P,7,KC]
                parts.append(_wrap16(np.transpose(blk, (1, 2, 0)).ravel()))
            blk = si[c, :, ts, :]  # [P, 7, KU2]
            parts.append(_wrap16(np.transpose(blk, (1, 2, 0)).ravel()))
            gidx[c, bi] = np.concatenate(parts, axis=1)

    deg_inv = (1.0 / np.maximum(deg, 1.0)).astype(np.float32)
    deginv_core = deg_inv[np.argsort(gid)].reshape(NCORES, TILES_PER_CORE, P)
    deginv_core = np.ascontiguousarray(np.transpose(deginv_core, (0, 2, 1)))

    xp = np.zeros((NP_ALL, FIN), np.float32)
    xp[gid[:N]] = np.asarray(x, np.float32)
    return dict(
        ku=int(ku),
        midx1=midx1,
        mloc1=mloc1,
        mloc2=mloc2,
        gidx=gidx,
        deginv=deginv_core,
        xp=xp,
        gid=gid,
    )


# ----------------------------------------------------------------------------
# device program
# ----------------------------------------------------------------------------
def _padP(a):
    out = np.zeros((P, a.shape[1]), a.dtype)
    out[: a.shape[0]] = a
    return out


def _hilo(v):
    hi = _bf16(np.asarray(v, np.float32))
    lo = _bf16(np.asarray(v, np.float32) - np.asarray(hi, np.float32))
    return hi, lo


def _raw_gather(nc, out_ap, in_ap, idxs_ap, num_idxs, elem_size, elem_step):
    """nc.gpsimd.dma_gather minus the elem_size_bytes%256 assert (HW-verified:
    only the row pitch needs 256B granularity). single_packet=False is
    mandatory for >128 descriptors (SWDGE ring depth)."""
    eng = nc.gpsimd
    dt_size = mybir.dt.size(in_ap.dtype)
    stride_bytes = elem_step * dt_size
    assert stride_bytes % 256 == 0 and stride_bytes // 256 < 256
    _in_ap = eng.lower_ap_dma(in_ap, for_custom_bir_dma=True)
    _idxs_ap = eng.lower_ap(idxs_ap)
    _out_ap = eng.lower_ap(out_ap)
    return eng.add_instruction(
        mybir.InstDMAGatherAnt(
            name=eng.bass.get_next_instruction_name(),
            ins=[*_in_ap, _idxs_ap, eng.lower_val_access(eng.to_reg(num_idxs))],
            outs=[_out_ap],
            transpose=False,
            num_idxs=num_idxs,
            elem_size=elem_size,
            stride_bytes_256=stride_bytes // 256,
            gen_mode=0,
            single_packet=False,
            queue_num=0,
            sbuf_tokens_per_rank=0,
            sbuf_free_dim_per_rank=0,
            sbuf_free_dim_pad_per_rank=0,
            sbuf_byte_offset=0,
        )
    )


def _build_program(ku):
    import os
    phases = int(os.environ.get("KERNEL_PHASES", "3"))
    nc = bacc.Bacc()
    cols1 = TILES_PER_CORE * ku
    cols2 = TILES_PER_CORE * KU2

    CF = 492
    CB = KU2 * P + cols1 + cols2
    xe_in = nc.declare_dram_parameter(
        "xe", [TILES_PER_CORE, P, ku * FIN], BF16, isOutput=False
    )
    xlocT = nc.declare_dram_parameter("xlocT", [FIN, NPC], F32, isOutput=False)
    cf_in = nc.declare_dram_parameter("constf", [P, CF], F32, isOutput=False)
    cb_in = nc.declare_dram_parameter("constb", [P, CB], BF16, isOutput=False)
    gidx_in = nc.declare_dram_parameter(
        "gidx", [NBATCH, P, GI], I16, isOutput=False
    )
    outloc = nc.declare_dram_parameter("outloc", [NPC, FOUT], F32, isOutput=True)

    # internal DRAM (z tables 256B-pitched for dma_gather)
    z1loc = nc.dram_tensor("z1loc", [NPC, FWP], BF16)
    z1tab = nc.dram_tensor("z1tab", [NP_ALL, FWP], BF16, addr_space="Shared")
    z2loc = nc.dram_tensor("z2loc", [NPC, FWP], BF16)
    z2tab = nc.dram_tensor("z2tab", [NP_ALL, FWP], BF16, addr_space="Shared")
    sd1 = nc.dram_tensor("sd1", [NPC, SDP], F32)
    sd2 = nc.dram_tensor("sd2", [NPC, SDP], F32)

    groups = [list(range(NCORES))]

    with tile.TileContext(nc) as tc:
        with (
            tc.tile_pool(name="const", bufs=1) as cpool,
            tc.tile_pool(name="sbuf", bufs=4) as pool,
            tc.tile_pool(name="gath", bufs=3) as gpool,
            tc.tile_pool(name="psum", bufs=2, space="PSUM") as pacc,
            tc.tile_pool(name="psum1", bufs=1, space="PSUM") as ptp,
        ):
            def cload(ap, shape, dt, tag):
                t = cpool.tile(shape, dt, tag=tag)
                nc.sync.dma_start(out=t[:], in_=ap)
                return t

            cf = cload(cf_in[:, :], [P, CF], F32, tag="cf")
            cb = cload(cb_in[:, :], [P, CB], BF16, tag="cb")
            ident_t = cf[:, 0:128]
            dinv_t = cf[:, 128:226]
            b2r_t = cf[:, 226:258]
            bout_t = cf[:HID, 258:259]
            b1c_t = cf[:HID, 259:260]
            a1_t = cf[:HID, 260:264]
            a2_t = cf[:FOUT, 264:268]
            wout_t = cf[:FIN, 268:332]
            wroot_t = cf[:FIN, 332:396]
            w1_t = cf[:HID, 396:460]
            w2_t = cf[:HID, 460:492]
            iotak_t = cb[:, 0 : KU2 * P]  # iota 0..127 tiled KU2 times
            mloc1_t = cb[:, KU2 * P : KU2 * P + cols1]
            mloc2_t = cb[:, KU2 * P + cols1 :]

            sdcol = cpool.tile([P, TILES_PER_CORE], F32, tag="sdcol")

            def sel_build(mloc_t, kk, ti):
                """0/1 bf16 selection [P, kk, P] for tile ti."""
                sel = pool.tile([P, kk, P], BF16, tag="sel")
                nc.vector.tensor_tensor(
                    out=sel[:, :, :],
                    in0=mloc_t[:, ti * kk : (ti + 1) * kk, None].to_broadcast(
                        [P, kk, P]
                    ),
                    in1=iotak_t[:, 0 : kk * P].rearrange("p (k c) -> p k c", k=kk),
                    op=OP.is_equal,
                )
                return sel

            def pack_from_T(hT_sb, w_t, a_t, fo, fw, zloc, ti):
                """Feature-major f32 activations hT_sb [fi, P] for tile ti ->
                z = h @ W, s_src/s_dst = z @ a; packed z-row to zloc rows,
                s_dst column stashed in sdcol."""
                zT_ps = ptp.tile([fo, P], F32, tag="zT")
                nc.tensor.matmul(
                    out=zT_ps[:], lhsT=w_t, rhs=hT_sb, start=True, stop=True
                )
                zT_sb = pool.tile([fo, P], F32, tag="zTsb")
                nc.scalar.copy(out=zT_sb[:], in_=zT_ps[:])
                sc_ps = ptp.tile([P, 4], F32, tag="sc")
                nc.tensor.matmul(
                    out=sc_ps[:], lhsT=zT_sb[:, :], rhs=a_t, start=True, stop=True
                )
                sc_sb = pool.tile([P, 4], F32, tag="sc_sb")
                nc.scalar.copy(out=sc_sb[:], in_=sc_ps[:, :])
                ssrc = pool.tile([P, 1], F32, tag="ssrc")
                nc.vector.tensor_tensor(
                    out=ssrc[:], in0=sc_sb[:, 0:1], in1=sc_sb[:, 1:2], op=OP.add
                )
                nc.vector.tensor_tensor(
                    out=sdcol[:, ti : ti + 1],
                    in0=sc_sb[:, 2:3],
                    in1=sc_sb[:, 3:4],
                    op=OP.add,
                )
                zr_ps = ptp.tile([P, fo], F32, tag="zr")
                nc.tensor.transpose(
                    out=zr_ps[:], in_=zT_sb[:, :], identity=ident_t[:fo, 0:fo]
                )
                zrow = pool.tile([P, fw], BF16, tag="zrow")
                nc.scalar.copy(out=zrow[:, 0:fo], in_=zr_ps[:, :])
                nc.vector.memset(zrow[:, fo : fo + 1], 1.0)
                nc.vector.memset(zrow[:, fo + 3 : fw], 0.0)
                nc.scalar.copy(out=zrow[:, fo + 1 : fo + 2], in_=ssrc[:, :])
                shi_f = pool.tile([P, 1], F32, tag="shif")
                nc.scalar.copy(out=shi_f[:], in_=zrow[:, fo + 1 : fo + 2])
                nc.vector.tensor_tensor(
                    out=zrow[:, fo + 2 : fo + 3],
                    in0=ssrc[:, :],
                    in1=shi_f[:, :],
                    op=OP.subtract,
                )
                nc.sync.dma_start(
                    out=zloc[ti * P : (ti + 1) * P, 0:fw], in_=zrow[:, :]
                )

            # ================= Layer 1: ClusterGCN =================
            for ti in range(TILES_PER_CORE):
                    msg = gpool.tile([P, ku * FIN], BF16, tag="msg1")
                    nc.sync.dma_start(out=msg[:, :], in_=xe_in[ti, :, :])
                    sel = sel_build(mloc1_t, ku, ti)
                    acc = pacc.tile([P, FIN], F32, tag="acc")
                    for k in range(ku):
                        nc.tensor.matmul(
                            out=acc[:],
                            lhsT=sel[:, k, :],
                            rhs=msg[:, k * FIN : (k + 1) * FIN],
                            start=(k == 0),
                            stop=(k == ku - 1),
                        )
                    agg = pool.tile([P, FIN], F32, tag="agg")
                    nc.scalar.mul(agg[:], acc[:, :], dinv_t[:, ti : ti + 1])
                    xT_sb = pool.tile([FIN, P], F32, tag="xT")
                    nc.sync.dma_start(
                        out=xT_sb[:], in_=xlocT[:, ti * P : (ti + 1) * P]
                    )
                    aT_ps = ptp.tile([FIN, P], F32, tag="tp")
                    nc.tensor.transpose(out=aT_ps[:], in_=agg[:, :], identity=ident_t)
                    aT_sb = pool.tile([FIN, P], F32, tag="aT")
                    nc.scalar.copy(out=aT_sb[:], in_=aT_ps[:])
                    hT_ps = ptp.tile([HID, P], F32, tag="hTp")
                    nc.tensor.matmul(
                        out=hT_ps[:], lhsT=wout_t, rhs=aT_sb[:, :],
                        start=True, stop=False,
                    )
                    nc.tensor.matmul(
                        out=hT_ps[:], lhsT=wroot_t, rhs=xT_sb[:, :],
                        start=False, stop=True,
                    )
                    h1T_sb = pool.tile([HID, P], F32, tag="h1T")
                    nc.scalar.activation(
                        out=h1T_sb[:], in_=hT_ps[:], func=AF.Relu, bias=bout_t
                    )
                    pack_from_T(h1T_sb[:, :], w1_t, a1_t, HID, FW1, z1loc, ti)
            nc.sync.dma_start(
                out=sd1[:, 0:1].rearrange("(p t) one -> p (t one)", p=P),
                in_=sdcol[:, :],
            )
            if phases >= 1:
                tc.strict_bb_all_engine_barrier()
                nc.gpsimd.collective_compute(
                    "AllGather",
                    OP.bypass,
                    replica_groups=groups,
                    ins=[z1loc[:, :]],
                    outs=[z1tab[:, :]],
                )
                tc.strict_bb_all_engine_barrier()

            # ================= Layers 2 & 3: GAT =================
            def gat_layer(ztab, zself, sdt, fw, fo, w_t, a_t, zloc_next, fw_next, sd_next, last):
                fz = fw - 4
                NZ16 = NIZ // 16  # 280
                for bi in range(NBATCH):
                    gix = gpool.tile([P, GI], I16, tag="gix")
                    nc.sync.dma_start(out=gix[:, :], in_=gidx_in[bi, :, :])
                    # z-row gathers: one per chunk (payload fw of FWP pitch)
                    msgq = gpool.tile([P, NCHUNK, BATCH * KC, fw], BF16, tag="msgq")
                    for ch in range(NCHUNK):
                        _raw_gather(
                            nc,
                            msgq[:, ch, :, :],
                            ztab[ch * CH : (ch + 1) * CH, 0:fw],
                            gix[:, ch * NZ16 : (ch + 1) * NZ16],
                            NIZ,
                            fw,
                            FWP,
                        )
                    # s_dst gather (4B payload of SDP-pitch table); split into
                    # 4 calls -- single calls above ~16k descriptors crash the
                    # Q7 (HW-observed), z gathers at 4480 are safe.
                    sde = gpool.tile([P, BATCH * KU2, 1], F32, tag="sde")
                    scols = BATCH * KU2  # 147
                    c0 = 0
                    for cc in (37, 37, 37, scols - 111):
                        _raw_gather(
                            nc,
                            sde[:, c0 : c0 + cc, :],
                            sdt[:, 0:1],
                            gix[
                                :,
                                NCHUNK * NZ16 + c0 * 8 : NCHUNK * NZ16
                                + (c0 + cc) * 8,
                            ],
                            cc * P,
                            1,
                            SDP,
                        )
                        c0 += cc
                    # self-loop rows: contiguous load from local zloc
                    selfb = gpool.tile([P, BATCH, fw], BF16, tag="selfb")
                    nc.sync.dma_start(
                        out=selfb[:, :, :],
                        in_=zself[
                            bi * BATCH * P : (bi + 1) * BATCH * P, 0:fw
                        ].rearrange("(b p) f -> p b f", p=P),
                    )
                    for b in range(BATCH):
                        ti = bi * BATCH + b
                        sel = sel_build(mloc2_t, KU2, ti)
                        js = slice(b * KC, (b + 1) * KC)
                        # logits l = s_src(hi+lo) + s_dst
                        l = pool.tile([P, KU2], F32, tag="l")
                        nc.vector.tensor_tensor(
                            out=l[:, 0:1],
                            in0=selfb[:, b, fz + 1 : fz + 2],
                            in1=selfb[:, b, fz + 2 : fz + 3],
                            op=OP.add,
                        )
                        nc.vector.tensor_tensor(
                            out=l[:, 1:KU2].rearrange(
                                "p (c j) -> p c j", c=NCHUNK
                            ),
                            in0=msgq[:, :, js, fz + 1],
                            in1=msgq[:, :, js, fz + 2],
                            op=OP.add,
                        )
                        nc.vector.tensor_tensor(
                            out=l[:],
                            in0=l[:],
                            in1=sde[:, b * KU2 : (b + 1) * KU2, 0],
                            op=OP.add,
                        )
                        # w = exp(leaky_relu(l)) = max(exp(l), exp(0.2*l))
                        w1e = pool.tile([P, KU2], F32, tag="w1")
                        nc.scalar.activation(out=w1e[:], in_=l[:], func=AF.Exp)
                        w2e = pool.tile([P, KU2], F32, tag="w2")
                        nc.scalar.activation(
                            out=w2e[:], in_=l[:], func=AF.Exp, scale=float(NEG)
                        )
                        wb = pool.tile([P, KU2], BF16, tag="wb")
                        nc.vector.tensor_tensor(
                            out=wb[:], in0=w1e[:], in1=w2e[:], op=OP.max
                        )
                        # weighted messages (+denominator column fz)
                        mp0 = pool.tile([P, fz + 1], BF16, tag="mp0")
                        nc.vector.tensor_tensor(
                            out=mp0[:],
                            in0=selfb[:, b, 0 : fz + 1],
                            in1=wb[:, 0:1].to_broadcast([P, fz + 1]),
                            op=OP.mult,
                        )
                        mpg = pool.tile(
                            [P, NCHUNK, KC, fz + 1], BF16, tag="mpg"
                        )
                        nc.vector.tensor_tensor(
                            out=mpg[:, :, :, :],
                            in0=msgq[:, :, js, 0 : fz + 1],
                            in1=wb[:, 1:KU2, None]
                            .rearrange("p (c j) one -> p c (j one)", c=NCHUNK)[
                                :, :, :, None
                            ]
                            .to_broadcast([P, NCHUNK, KC, fz + 1]),
                            op=OP.mult,
                        )
                        acc = pacc.tile([P, fz + 1], F32, tag="acc")
                        nc.tensor.matmul(
                            out=acc[:], lhsT=sel[:, 0, :], rhs=mp0[:, :],
                            start=True, stop=False,
                        )
                        for ch in range(NCHUNK):
                            for j in range(KC):
                                nc.tensor.matmul(
                                    out=acc[:],
                                    lhsT=sel[:, 1 + ch * KC + j, :],
                                    rhs=mpg[:, ch, j, :],
                                    start=False,
                                    stop=(ch == NCHUNK - 1 and j == KC - 1),
                                )
                        den = pool.tile([P, 1], F32, tag="den")
                        nc.vector.tensor_scalar(
                            out=den[:], in0=acc[:, fz : fz + 1], scalar1=1e-30,
                            scalar2=None, op0=OP.max,
                        )
                        rec = pool.tile([P, 1], F32, tag="rec")
                        nc.vector.reciprocal(out=rec[:], in_=den[:])
                        if last:
                            h = pool.tile([P, fz], F32, tag="h")
                            nc.scalar.mul(h[:], acc[:, 0:fz], rec[:, :])
                            nc.vector.tensor_tensor(
                                out=h[:], in0=h[:], in1=b2r_t, op=OP.add
                            )
                            nc.sync.dma_start(
                                out=outloc[ti * P : (ti + 1) * P, :], in_=h[:, :]
                            )
                        else:
                            hsc = pool.tile([P, fz], F32, tag="hsc")
                            nc.scalar.mul(hsc[:], acc[:, 0:fz], rec[:, :])
                            hT_ps = ptp.tile([fz, P], F32, tag="tp")
                            nc.tensor.transpose(
                                out=hT_ps[:], in_=hsc[:, :], identity=ident_t
                            )
                            hT_sb = pool.tile([fz, P], F32, tag="h1T")
                            nc.scalar.activation(
                                out=hT_sb[:], in_=hT_ps[:], func=AF.Relu,
                                bias=b1c_t,
                            )
                            pack_from_T(
                                hT_sb[:, :], w_t, a_t, fw_next - 4, fw_next,
                                zloc_next, ti,
                            )
                if not last:
                    nc.sync.dma_start(
                        out=sd_next[:, 0:1].rearrange("(p t) one -> p (t one)", p=P),
                        in_=sdcol[:, :],
                    )

            if phases >= 2:
                gat_layer(z1tab, z1loc, sd1, FW1, HID, w2_t, a2_t, z2loc, FW2, sd2, False)
            if phases >= 3:
                tc.strict_bb_all_engine_barrier()
                nc.gpsimd.collective_compute(
                    "AllGather",
                    OP.bypass,
                    replica_groups=groups,
                    ins=[z2loc[:, :]],
                    outs=[z2tab[:, :]],
                )
                tc.strict_bb_all_engine_barrier()
                gat_layer(z2tab, z2loc, sd2, FW2, FOUT, None, None, None, None, None, True)
            if phases < 3:
                for ti in range(TILES_PER_CORE):
                    zt = pool.tile([P, FOUT], F32, tag="h")
                    nc.vector.memset(zt[:], 0.0)
                    nc.sync.dma_start(out=outloc[ti * P : (ti + 1) * P, :], in_=zt[:, :])

    nc.finalize()
    return nc


# ----------------------------------------------------------------------------
# entry point
# ----------------------------------------------------------------------------
def kernel(
    x,
    edge_index,
    W_out,
    b_out,
    W_root,
    W1,
    a_src1,
    a_dst1,
    b1,
    W2,
    a_src2,
    a_dst2,
    b2,
    training=0,
    **_unused,
):
    pre = _preprocess(x, edge_index)
    ku = pre["ku"]
    import os as _os
    _key = (ku, _os.environ.get("KERNEL_PHASES", "3"))
    if _key not in _cache:
        _cache[_key] = _build_program(ku)
    nc = _cache[_key]

    iota = np.tile(np.arange(P, dtype=np.float32), (P, 1))
    iotak = np.tile(iota, (1, KU2))
    ident = np.eye(P, dtype=np.float32)

    a1hi, a1lo = _hilo(np.asarray(a_src1, np.float32))
    a1dhi, a1dlo = _hilo(np.asarray(a_dst1, np.float32))
    a2hi, a2lo = _hilo(np.asarray(a_src2, np.float32))
    a2dhi, a2dlo = _hilo(np.asarray(a_dst2, np.float32))
    a1 = np.stack(
        [np.float32(a1hi), np.float32(a1lo), np.float32(a1dhi), np.float32(a1dlo)], 1
    )
    a2 = np.stack(
        [np.float32(a2hi), np.float32(a2lo), np.float32(a2dhi), np.float32(a2dlo)], 1
    )

    xp = pre["xp"]
    xp_b = _bf16(xp)
    in_maps = []
    for c in range(NCORES):
        mi = pre["midx1"][c].reshape(P, TILES_PER_CORE, ku).astype(np.int64)
        xe = np.ascontiguousarray(
            np.transpose(xp_b[mi], (1, 0, 2, 3))
        ).reshape(TILES_PER_CORE, P, ku * FIN)
        in_maps.append(
            {
                "xe": xe,
                "xlocT": np.ascontiguousarray(xp[c * NPC : (c + 1) * NPC].T),
                "constf": np.concatenate(
                    [
                        ident,
                        pre["deginv"][c],
                        np.tile(np.asarray(b2, np.float32), (P, 1)),
                        _padP(np.asarray(b_out, np.float32).reshape(HID, 1)),
                        _padP(np.asarray(b1, np.float32).reshape(HID, 1)),
                        _padP(a1),
                        _padP(a2),
                        _padP(np.asarray(W_out, np.float32)),
                        _padP(np.asarray(W_root, np.float32)),
                        _padP(np.asarray(W1, np.float32)),
                        _padP(np.asarray(W2, np.float32)),
                    ],
                    axis=1,
                ),
                "constb": np.concatenate(
                    [_bf16(iotak), _bf16(pre["mloc1"][c]), _bf16(pre["mloc2"][c])],
                    axis=1,
                ),
                "gidx": pre["gidx"][c],
            }
        )

    import os
    trace = bool(os.environ.get("BASS_TRACE"))
    res = run_bass_kernel_spmd(
        nc, in_maps, list(range(NCORES)), trace=trace
    )
    global last_result
    last_result = res
    out_p = np.concatenate([res.results[c]["outloc"] for c in range(NCORES)], 0)
    out = out_p[pre["gid"][:N]]
    return np.asarray(out, np.float32)


# revision 10
# speedup vs baseline: 1.7468x; 1.7468x over previous
"""ClusterGCN + 2x GAT message-passing kernel for 8 Trainium2 NeuronCores.

Strategy (dst-sharded, one SPMD program):
  - Nodes are permuted into 784 tiles of 128 slots, load-balanced so every
    tile has (nearly) the same number of incoming edges (self-loops added).
    Cores own 98 consecutive tiles each.
  - Per layer, each core gathers the rows of its incoming messages from a
    replicated node table in its HBM via per-column indirect DMA (the Q7
    descriptor-generation rate is ~8ns/row regardless of batching, so the
    gather is organized for zero padding), reduces them per dst tile with
    0/1 selection-matrix matmuls accumulated in PSUM, and applies the layer
    transform in feature-major (transposed) space.
  - Per-edge s_dst: the per-tile s_dst column is row-replicated with one PE
    transpose-of-broadcast (tile_scatter_add trick), then expanded to edge
    positions by reusing the selection matrix: sde2 = reduce_c(sel * sdfull)
    on the Vector engine. No extra DMA descriptors, no per-column selT
    transposes/matmuls.
  - exp(leaky_relu(l)) = max(exp(l), exp(0.2*l)): both exps on the Scalar
    engine (scale fused). s_src rides in the gathered row as bf16 hi/lo.
  - PSUM evacuation / scaling copies run on the Scalar engine.
  - Between layers the per-core z-tables (h @ W with packed attention
    scalars) are AllGathered so every core can gather arbitrary source rows.
"""

import os
import sys

sys.path.insert(0, "/opt/trn_rl_repo")
os.environ.setdefault("NEURON_RT_RESET_CORES", "1")

import numpy as np

import concourse.bacc as bacc
import concourse.bass as bass
import concourse.mybir as mybir
import concourse.tile as tile
from concourse.bass_utils import run_bass_kernel_spmd

# ---- problem constants (hardcoded per contest rules) ----
N = 100000
E = 1600000
FIN = 64
HID = 64
FOUT = 32
NEG = 0.2

P = 128
NCORES = 8
TILES_PER_CORE = 98
T_ALL = NCORES * TILES_PER_CORE  # 784
NPC = TILES_PER_CORE * P  # 12544 nodes per core
NP_ALL = T_ALL * P  # 100352 padded node count

FW1 = 68  # z1 row: z(64) | 1.0 | s_hi | s_lo | pad
FW2 = 36  # z2 row: z(32) | 1.0 | s_hi | s_lo | pad

F32 = mybir.dt.float32
BF16 = mybir.dt.bfloat16
I32 = mybir.dt.int32
AF = mybir.ActivationFunctionType
OP = mybir.AluOpType

_cache = {}
last_result = None


def _bf16(a):
    import ml_dtypes

    return np.asarray(a, dtype=ml_dtypes.bfloat16)


# ----------------------------------------------------------------------------
# host-side preprocessing
# ----------------------------------------------------------------------------
def _balance_tiles(deg):
    """Assign each of NP_ALL nodes to one of T_ALL tiles (128 slots each) so
    that per-tile total in-degree is near-uniform. Returns perm arrays."""
    import heapq

    order = np.argsort(-deg, kind="stable")
    heap = [(0, t) for t in range(T_ALL)]
    heapq.heapify(heap)
    counts = np.zeros(T_ALL, np.int64)
    loads = np.zeros(T_ALL, np.int64)
    tile_of = np.empty(NP_ALL, np.int32)
    slot_of = np.empty(NP_ALL, np.int32)
    for n in order:
        while True:
            load, t = heapq.heappop(heap)
            if counts[t] < P:
                break
        tile_of[n] = t
        slot_of[n] = counts[t]
        counts[t] += 1
        loads[t] += deg[n]
        if counts[t] < P:
            heapq.heappush(heap, (loads[t], t))
    return tile_of, slot_of, int(loads.max())


def _preprocess(x, edge_index):
    src = np.asarray(edge_index[0], np.int64)
    dst = np.asarray(edge_index[1], np.int64)
    loops = np.arange(NP_ALL, dtype=np.int64)
    src_all = np.concatenate([src, loops])
    dst_all = np.concatenate([dst, loops])
    deg = np.bincount(dst_all, minlength=NP_ALL)  # includes self-loop

    tile_of, slot_of, max_load = _balance_tiles(deg)
    ku = (max_load + P - 1) // P
    gid = tile_of.astype(np.int64) * P + slot_of  # node -> permuted row

    m_src = gid[src_all]
    m_tile = tile_of[dst_all].astype(np.int64)
    m_slot = slot_of[dst_all].astype(np.int64)

    # column 0 of every tile holds the 128 self-loop messages slot-aligned;
    # edge messages fill columns 1..ku-1
    is_loop_m = np.concatenate(
        [np.zeros(len(src), bool), np.ones(NP_ALL, bool)]
    )
    order = np.lexsort((~is_loop_m, m_tile))
    m_src, m_tile, m_slot, is_loop_m = (
        m_src[order],
        m_tile[order],
        m_slot[order],
        is_loop_m[order],
    )
    tile_counts = np.bincount(m_tile, minlength=T_ALL)
    tile_starts = np.concatenate([[0], np.cumsum(tile_counts)[:-1]])
    pos = np.arange(len(m_src)) - tile_starts[m_tile]
    mp = np.where(is_loop_m, m_slot, pos % P)
    mc = np.where(is_loop_m, 0, pos // P)

    cols = TILES_PER_CORE * ku
    midx = np.zeros((NCORES, P, cols), np.int32)
    mloc = np.full((NCORES, P, cols), -1.0, np.float32)
    core = m_tile // TILES_PER_CORE
    tl = m_tile % TILES_PER_CORE
    col = tl * ku + mc
    midx[core, mp, col] = m_src
    mloc[core, mp, col] = m_slot

    deg_inv = (1.0 / np.maximum(deg, 1.0)).astype(np.float32)
    deginv_core = deg_inv[np.argsort(gid)].reshape(NCORES, TILES_PER_CORE, P)
    deginv_core = np.ascontiguousarray(np.transpose(deginv_core, (0, 2, 1)))

    xp = np.zeros((NP_ALL, FIN), np.float32)
    xp[gid[:N]] = np.asarray(x, np.float32)
    return dict(
        ku=int(ku),
        midx=midx,
        mloc=mloc,
        deginv=deginv_core,
        xp=xp,
        gid=gid,
    )


# ----------------------------------------------------------------------------
# device program
# ----------------------------------------------------------------------------
def _padP(a):
    out = np.zeros((P, a.shape[1]), a.dtype)
    out[: a.shape[0]] = a
    return out


def _hilo(v):
    hi = _bf16(np.asarray(v, np.float32))
    lo = _bf16(np.asarray(v, np.float32) - np.asarray(hi, np.float32))
    return hi, lo


def _build_program(ku):
    phases = int(os.environ.get("KERNEL_PHASES", "3"))
    nc = bacc.Bacc()
    cols = TILES_PER_CORE * ku

    CF = 492
    CB = ku * P + cols
    CI = cols
    xe_in = nc.declare_dram_parameter(
        "xe", [TILES_PER_CORE, P, ku * FIN], BF16, isOutput=False
    )
    xlocT = nc.declare_dram_parameter("xlocT", [FIN, NPC], F32, isOutput=False)
    cf_in = nc.declare_dram_parameter("constf", [P, CF], F32, isOutput=False)
    cb_in = nc.declare_dram_parameter("constb", [P, CB], BF16, isOutput=False)
    ci_in = nc.declare_dram_parameter("consti", [P, CI], I32, isOutput=False)
    outloc = nc.declare_dram_parameter("outloc", [NPC, FOUT], F32, isOutput=True)

    z1loc = nc.dram_tensor("z1loc", [NPC, FW1], BF16)
    z1tab = nc.dram_tensor("z1tab", [NP_ALL, FW1], BF16, addr_space="Shared")
    z2loc = nc.dram_tensor("z2loc", [NPC, FW2], BF16)
    z2tab = nc.dram_tensor("z2tab", [NP_ALL, FW2], BF16, addr_space="Shared")
    sd1 = nc.dram_tensor("sd1", [NPC, 1], F32)
    sd2 = nc.dram_tensor("sd2", [NPC, 1], F32)

    groups = [list(range(NCORES))]

    with tile.TileContext(nc) as tc:
        with (
            tc.tile_pool(name="const", bufs=1) as cpool,
            tc.tile_pool(name="sbuf", bufs=4) as pool,
            tc.tile_pool(name="gath", bufs=6) as gpool,
            tc.tile_pool(name="psum", bufs=2, space="PSUM") as pacc,
            tc.tile_pool(name="psum1", bufs=1, space="PSUM") as ptp,
        ):
            def cload(ap, shape, dt, tag):
                t = cpool.tile(shape, dt, tag=tag)
                nc.sync.dma_start(out=t[:], in_=ap)
                return t

            cf = cload(cf_in[:, :], [P, CF], F32, tag="cf")
            cb = cload(cb_in[:, :], [P, CB], BF16, tag="cb")
            ci = cload(ci_in[:, :], [P, CI], I32, tag="ci")
            ident_t = cf[:, 0:128]
            dinv_t = cf[:, 128:226]
            b2r_t = cf[:, 226:258]
            bout_t = cf[:HID, 258:259]
            b1c_t = cf[:HID, 259:260]
            a1_t = cf[:HID, 260:264]
            a2_t = cf[:FOUT, 264:268]
            wout_t = cf[:FIN, 268:332]
            wroot_t = cf[:FIN, 332:396]
            w1_t = cf[:HID, 396:460]
            w2_t = cf[:HID, 460:492]
            iotak_t = cb[:, 0 : ku * P]
            mloc_t = cb[:, ku * P :]
            midx_t = ci[:, :]

            sdcol = cpool.tile([P, TILES_PER_CORE], F32, tag="sdcol")

            def sel_build(ti):
                """0/1 bf16 selection [P, ku, P] for tile ti."""
                sel = pool.tile([P, ku, P], BF16, tag="sel")
                nc.vector.tensor_tensor(
                    out=sel[:, :, :],
                    in0=mloc_t[:, ti * ku : (ti + 1) * ku, None].to_broadcast(
                        [P, ku, P]
                    ),
                    in1=iotak_t[:].rearrange("p (k c) -> p k c", k=ku),
                    op=OP.is_equal,
                )
                return sel

            def pack_from_T(hT_sb, w_t, a_t, fo, fw, zloc, ti):
                """Feature-major f32 activations hT_sb [fi, P] for tile ti ->
                z = h @ W, s_src / s_dst = z @ a, packed z-row to zloc,
                s_dst column stashed in sdcol."""
                zT_ps = ptp.tile([fo, P], F32, tag="zT")
                nc.tensor.matmul(
                    out=zT_ps[:], lhsT=w_t, rhs=hT_sb, start=True, stop=True
                )
                zT_sb = pool.tile([fo, P], F32, tag="zTsb")
                nc.scalar.copy(out=zT_sb[:], in_=zT_ps[:])
                sc_ps = ptp.tile([P, 4], F32, tag="sc")
                nc.tensor.matmul(
                    out=sc_ps[:], lhsT=zT_sb[:, :], rhs=a_t, start=True, stop=True
                )
                sc_sb = pool.tile([P, 4], F32, tag="sc_sb")
                nc.scalar.copy(out=sc_sb[:], in_=sc_ps[:, :])
                ssrc = pool.tile([P, 1], F32, tag="ssrc")
                nc.vector.tensor_tensor(
                    out=ssrc[:], in0=sc_sb[:, 0:1], in1=sc_sb[:, 1:2], op=OP.add
                )
                nc.vector.tensor_tensor(
                    out=sdcol[:, ti : ti + 1],
                    in0=sc_sb[:, 2:3],
                    in1=sc_sb[:, 3:4],
                    op=OP.add,
                )
                zr_ps = ptp.tile([P, fo], F32, tag="zr")
                nc.tensor.transpose(
                    out=zr_ps[:], in_=zT_sb[:, :], identity=ident_t[:fo, 0:fo]
                )
                zrow = pool.tile([P, fw], BF16, tag="zrow")
                nc.scalar.copy(out=zrow[:, 0:fo], in_=zr_ps[:, :])
                nc.vector.memset(zrow[:, fo : fo + 1], 1.0)
                nc.vector.memset(zrow[:, fo + 3 : fw], 0.0)
                nc.scalar.copy(out=zrow[:, fo + 1 : fo + 2], in_=ssrc[:, :])
                shi_f = pool.tile([P, 1], F32, tag="shif")
                nc.scalar.copy(out=shi_f[:], in_=zrow[:, fo + 1 : fo + 2])
                nc.vector.tensor_tensor(
                    out=zrow[:, fo + 2 : fo + 3],
                    in0=ssrc[:, :],
                    in1=shi_f[:, :],
                    op=OP.subtract,
                )
                nc.sync.dma_start(
                    out=zloc[ti * P : (ti + 1) * P, :], in_=zrow[:, :]
                )

            # ================= Layer 1: ClusterGCN =================
            for ti in range(TILES_PER_CORE):
                    msg = gpool.tile([P, ku * FIN], BF16, tag="msg1")
                    nc.sync.dma_start(out=msg[:, :], in_=xe_in[ti, :, :])
                    sel = sel_build(ti)
                    acc = pacc.tile([P, FIN], F32, tag="acc")
                    for k in range(ku):
                        nc.tensor.matmul(
                            out=acc[:],
                            lhsT=sel[:, k, :],
                            rhs=msg[:, k * FIN : (k + 1) * FIN],
                            start=(k == 0),
                            stop=(k == ku - 1),
                        )
                    agg = pool.tile([P, FIN], F32, tag="agg")
                    nc.scalar.mul(agg[:], acc[:, :], dinv_t[:, ti : ti + 1])
                    xT_sb = pool.tile([FIN, P], F32, tag="xT")
                    nc.sync.dma_start(
                        out=xT_sb[:], in_=xlocT[:, ti * P : (ti + 1) * P]
                    )
                    aT_ps = ptp.tile([FIN, P], F32, tag="tp")
                    nc.tensor.transpose(out=aT_ps[:], in_=agg[:, :], identity=ident_t)
                    aT_sb = pool.tile([FIN, P], F32, tag="aT")
                    nc.scalar.copy(out=aT_sb[:], in_=aT_ps[:])
                    hT_ps = ptp.tile([HID, P], F32, tag="hTp")
                    nc.tensor.matmul(
                        out=hT_ps[:], lhsT=wout_t, rhs=aT_sb[:, :],
                        start=True, stop=False,
                    )
                    nc.tensor.matmul(
                        out=hT_ps[:], lhsT=wroot_t, rhs=xT_sb[:, :],
                        start=False, stop=True,
                    )
                    h1T_sb = pool.tile([HID, P], F32, tag="h1T")
                    nc.scalar.activation(
                        out=h1T_sb[:], in_=hT_ps[:], func=AF.Relu, bias=bout_t
                    )
                    pack_from_T(h1T_sb[:, :], w1_t, a1_t, HID, FW1, z1loc, ti)
            nc.sync.dma_start(
                out=sd1[:, :].rearrange("(p t) one -> p (t one)", p=P),
                in_=sdcol[:, :],
            )
            if phases >= 1:
                tc.strict_bb_all_engine_barrier()
                nc.gpsimd.collective_compute(
                    "AllGather",
                    OP.bypass,
                    replica_groups=groups,
                    ins=[z1loc[:, :]],
                    outs=[z1tab[:, :]],
                )
                tc.strict_bb_all_engine_barrier()

            # ================= Layers 2 & 3: GAT =================
            def gat_layer(ztab, zself, sdt, fw, fo, w_t, a_t, zloc_next, fw_next, sd_next, last):
                fz = fw - 4
                sdl = cpool.tile([P, TILES_PER_CORE], F32, tag=f"sdl{fw}")
                nc.sync.dma_start(
                    out=sdl[:],
                    in_=sdt[:, :].rearrange("(p t) one -> p (t one)", p=P),
                )
                for ti in range(TILES_PER_CORE):
                        msg = gpool.tile([P, ku, fw], BF16, tag="msg2")
                        # column 0 = slot-aligned self-loop rows: contiguous
                        nc.sync.dma_start(
                            out=msg[:, 0, :],
                            in_=zself[ti * P : (ti + 1) * P, :],
                        )
                        for k in range(1, ku):
                            nc.gpsimd.indirect_dma_start(
                                out=msg[:, k, :],
                                out_offset=None,
                                in_=ztab[:, :],
                                in_offset=bass.IndirectOffsetOnAxis(
                                    ap=midx_t[:, ti * ku + k : ti * ku + k + 1],
                                    axis=0,
                                ),
                            )
                        sel = sel_build(ti)
                        # s_dst row-replication: transpose of broadcast column
                        sdf_ps = ptp.tile([P, P], F32, tag="sdf")
                        nc.tensor.transpose(
                            out=sdf_ps[:],
                            in_=sdl[:, ti : ti + 1].to_broadcast([P, P]),
                            identity=ident_t,
                        )
                        sdf_sb = pool.tile([P, P], BF16, tag="sdfb")
                        nc.scalar.copy(out=sdf_sb[:], in_=sdf_ps[:])
                        # sde2[p,k] = sum_c sel[p,k,c] * s_dst[c]
                        sds = pool.tile([P, ku, P], BF16, tag="sds")
                        nc.vector.tensor_tensor(
                            out=sds[:, :, :],
                            in0=sel[:, :, :],
                            in1=sdf_sb[:, None, :].to_broadcast([P, ku, P]),
                            op=OP.mult,
                        )
                        sde2 = pool.tile([P, ku], F32, tag="sde2")
                        nc.vector.tensor_reduce(
                            out=sde2[:, :],
                            in_=sds[:, :, :],
                            axis=mybir.AxisListType.X,
                            op=OP.add,
                        )
                        # logits l = s_src(hi+lo) + s_dst
                        l = pool.tile([P, ku], F32, tag="l")
                        nc.vector.tensor_tensor(
                            out=l[:],
                            in0=msg[:, :, fz + 1],
                            in1=msg[:, :, fz + 2],
                            op=OP.add,
                        )
                        nc.vector.tensor_tensor(
                            out=l[:], in0=l[:], in1=sde2[:, :], op=OP.add
                        )
                        # w = exp(leaky_relu(l)) = max(exp(l), exp(0.2*l))
                        w1e = pool.tile([P, ku], F32, tag="w1")
                        nc.scalar.activation(out=w1e[:], in_=l[:], func=AF.Exp)
                        w2e = pool.tile([P, ku], F32, tag="w2")
                        nc.scalar.activation(
                            out=w2e[:], in_=l[:], func=AF.Exp, scale=float(NEG)
                        )
                        wb = pool.tile([P, ku], BF16, tag="wb")
                        nc.vector.tensor_tensor(
                            out=wb[:], in0=w1e[:], in1=w2e[:], op=OP.max
                        )
                        # weighted messages (+denominator column fz)
                        mp = pool.tile([P, ku, fz + 1], BF16, tag="mp")
                        nc.vector.tensor_tensor(
                            out=mp[:, :, :],
                            in0=msg[:, :, 0 : fz + 1],
                            in1=wb[:, :, None].to_broadcast([P, ku, fz + 1]),
                            op=OP.mult,
                        )
                        acc = pacc.tile([P, fz + 1], F32, tag="acc")
                        for k in range(ku):
                            nc.tensor.matmul(
                                out=acc[:],
                                lhsT=sel[:, k, :],
                                rhs=mp[:, k, :],
                                start=(k == 0),
                                stop=(k == ku - 1),
                            )
                        den = pool.tile([P, 1], F32, tag="den")
                        nc.vector.tensor_scalar(
                            out=den[:], in0=acc[:, fz : fz + 1], scalar1=1e-30,
                            scalar2=None, op0=OP.max,
                        )
                        rec = pool.tile([P, 1], F32, tag="rec")
                        nc.vector.reciprocal(out=rec[:], in_=den[:])
                        if last:
                            h = pool.tile([P, fz], F32, tag="h")
                            nc.scalar.mul(h[:], acc[:, 0:fz], rec[:, :])
                            nc.vector.tensor_tensor(
                                out=h[:], in0=h[:], in1=b2r_t, op=OP.add
                            )
                            nc.sync.dma_start(
                                out=outloc[ti * P : (ti + 1) * P, :], in_=h[:, :]
                            )
                        else:
                            hsc = pool.tile([P, fz], F32, tag="hsc")
                            nc.scalar.mul(hsc[:], acc[:, 0:fz], rec[:, :])
                            hT_ps = ptp.tile([fz, P], F32, tag="tp")
                            nc.tensor.transpose(
                                out=hT_ps[:], in_=hsc[:, :], identity=ident_t
                            )
                            hT_sb = pool.tile([fz, P], F32, tag="h1T")
                            nc.scalar.activation(
                                out=hT_sb[:], in_=hT_ps[:], func=AF.Relu,
                                bias=b1c_t,
                            )
                            pack_from_T(
                                hT_sb[:, :], w_t, a_t, fw_next - 4, fw_next,
                                zloc_next, ti,
                            )
                if not last:
                    nc.sync.dma_start(
                        out=sd_next[:, :].rearrange("(p t) one -> p (t one)", p=P),
                        in_=sdcol[:, :],
                    )

            if phases >= 2:
                gat_layer(z1tab, z1loc, sd1, FW1, HID, w2_t, a2_t, z2loc, FW2, sd2, False)
            if phases >= 3:
                tc.strict_bb_all_engine_barrier()
                nc.gpsimd.collective_compute(
                    "AllGather",
                    OP.bypass,
                    replica_groups=groups,
                    ins=[z2loc[:, :]],
                    outs=[z2tab[:, :]],
                )
                tc.strict_bb_all_engine_barrier()
                gat_layer(z2tab, z2loc, sd2, FW2, FOUT, None, None, None, None, None, True)
            if phases < 3:
                for ti in range(TILES_PER_CORE):
                    zt = pool.tile([P, FOUT], F32, tag="h")
                    nc.vector.memset(zt[:], 0.0)
                    nc.sync.dma_start(out=outloc[ti * P : (ti + 1) * P, :], in_=zt[:, :])

    nc.finalize()
    return nc


# ----------------------------------------------------------------------------
# entry point
# ----------------------------------------------------------------------------
def kernel(
    x,
    edge_index,
    W_out,
    b_out,
    W_root,
    W1,
    a_src1,
    a_dst1,
    b1,
    W2,
    a_src2,
    a_dst2,
    b2,
    training=0,
    **_unused,
):
    pre = _preprocess(x, edge_index)
    ku = pre["ku"]
    _key = (ku, os.environ.get("KERNEL_PHASES", "3"))
    if _key not in _cache:
        _cache[_key] = _build_program(ku)
    nc = _cache[_key]

    iota = np.tile(np.arange(P, dtype=np.float32), (P, 1))
    iotak = np.tile(iota, (1, ku))
    ident = np.eye(P, dtype=np.float32)

    a1hi, a1lo = _hilo(np.asarray(a_src1, np.float32))
    a1dhi, a1dlo = _hilo(np.asarray(a_dst1, np.float32))
    a2hi, a2lo = _hilo(np.asarray(a_src2, np.float32))
    a2dhi, a2dlo = _hilo(np.asarray(a_dst2, np.float32))
    a1 = np.stack(
        [np.float32(a1hi), np.float32(a1lo), np.float32(a1dhi), np.float32(a1dlo)], 1
    )
    a2 = np.stack(
        [np.float32(a2hi), np.float32(a2lo), np.float32(a2dhi), np.float32(a2dlo)], 1
    )

    xp = pre["xp"]
    xp_b = _bf16(xp)
    in_maps = []
    for c in range(NCORES):
        mi = pre["midx"][c].reshape(P, TILES_PER_CORE, ku).astype(np.int64)
        xe = np.ascontiguousarray(
            np.transpose(xp_b[mi], (1, 0, 2, 3))
        ).reshape(TILES_PER_CORE, P, ku * FIN)
        in_maps.append(
            {
                "xe": xe,
                "xlocT": np.ascontiguousarray(xp[c * NPC : (c + 1) * NPC].T),
                "constf": np.concatenate(
                    [
                        ident,
                        pre["deginv"][c],
                        np.tile(np.asarray(b2, np.float32), (P, 1)),
                        _padP(np.asarray(b_out, np.float32).reshape(HID, 1)),
                        _padP(np.asarray(b1, np.float32).reshape(HID, 1)),
                        _padP(a1),
                        _padP(a2),
                        _padP(np.asarray(W_out, np.float32)),
                        _padP(np.asarray(W_root, np.float32)),
                        _padP(np.asarray(W1, np.float32)),
                        _padP(np.asarray(W2, np.float32)),
                    ],
                    axis=1,
                ),
                "constb": np.concatenate(
                    [_bf16(iotak), _bf16(pre["mloc"][c])], axis=1
                ),
                "consti": pre["midx"][c],
            }
        )

    trace = bool(os.environ.get("BASS_TRACE"))
    res = run_bass_kernel_spmd(
        nc, in_maps, list(range(NCORES)), trace=trace
    )
    global last_result
    last_result = res
    out_p = np.concatenate([res.results[c]["outloc"] for c in range(NCORES)], 0)
    out = out_p[pre["gid"][:N]]
    return np.asarray(out, np.float32)


# revision 13
# speedup vs baseline: 1.7512x; 1.0025x over previous
"""ClusterGCN + 2x GAT message-passing kernel for 8 Trainium2 NeuronCores.

Strategy (dst-sharded, one SPMD program):
  - Nodes are permuted into 784 tiles of 128 slots, load-balanced so every
    tile has (nearly) the same number of incoming edges (self-loops added).
    Cores own 98 consecutive tiles each.
  - Per layer, each core gathers the rows of its incoming messages from a
    replicated node table in its HBM via per-column indirect DMA (the Q7
    descriptor-generation rate is ~8ns/row regardless of batching, so the
    gather is organized for zero padding), reduces them per dst tile with
    0/1 selection-matrix matmuls accumulated in PSUM, and applies the layer
    transform in feature-major (transposed) space.
  - Per-edge s_dst: the per-tile s_dst column is row-replicated with one PE
    transpose-of-broadcast (tile_scatter_add trick), then expanded to edge
    positions by reusing the selection matrix: sde2 = reduce_c(sel * sdfull)
    on the Vector engine. No extra DMA descriptors, no per-column selT
    transposes/matmuls.
  - exp(leaky_relu(l)) = max(exp(l), exp(0.2*l)): both exps on the Scalar
    engine (scale fused). s_src rides in the gathered row as bf16 hi/lo.
  - PSUM evacuation / scaling copies run on the Scalar engine.
  - Between layers the per-core z-tables (h @ W with packed attention
    scalars) are AllGathered so every core can gather arbitrary source rows.
"""

import os
import sys

sys.path.insert(0, "/opt/trn_rl_repo")
os.environ.setdefault("NEURON_RT_RESET_CORES", "1")

import numpy as np

import concourse.bacc as bacc
import concourse.bass as bass
import concourse.mybir as mybir
import concourse.tile as tile
from concourse.bass_utils import run_bass_kernel_spmd

# ---- problem constants (hardcoded per contest rules) ----
N = 100000
E = 1600000
FIN = 64
HID = 64
FOUT = 32
NEG = 0.2

P = 128
NCORES = 8
TILES_PER_CORE = 98
T_ALL = NCORES * TILES_PER_CORE  # 784
NPC = TILES_PER_CORE * P  # 12544 nodes per core
NP_ALL = T_ALL * P  # 100352 padded node count

FW1 = 68  # z1 row: z(64) | 1.0 | s_hi | s_lo | pad
FW2 = 36  # z2 row: z(32) | 1.0 | s_hi | s_lo | pad

F32 = mybir.dt.float32
BF16 = mybir.dt.bfloat16
I32 = mybir.dt.int32
AF = mybir.ActivationFunctionType
OP = mybir.AluOpType

_cache = {}
last_result = None


def _bf16(a):
    import ml_dtypes

    return np.asarray(a, dtype=ml_dtypes.bfloat16)


# ----------------------------------------------------------------------------
# host-side preprocessing
# ----------------------------------------------------------------------------
def _balance_tiles(deg):
    """Assign each of NP_ALL nodes to one of T_ALL tiles (128 slots each) so
    that per-tile total in-degree is near-uniform. Returns perm arrays."""
    import heapq

    order = np.argsort(-deg, kind="stable")
    heap = [(0, t) for t in range(T_ALL)]
    heapq.heapify(heap)
    counts = np.zeros(T_ALL, np.int64)
    loads = np.zeros(T_ALL, np.int64)
    tile_of = np.empty(NP_ALL, np.int32)
    slot_of = np.empty(NP_ALL, np.int32)
    for n in order:
        while True:
            load, t = heapq.heappop(heap)
            if counts[t] < P:
                break
        tile_of[n] = t
        slot_of[n] = counts[t]
        counts[t] += 1
        loads[t] += deg[n]
        if counts[t] < P:
            heapq.heappush(heap, (loads[t], t))
    return tile_of, slot_of, int(loads.max())


def _preprocess(x, edge_index):
    src = np.asarray(edge_index[0], np.int64)
    dst = np.asarray(edge_index[1], np.int64)
    loops = np.arange(NP_ALL, dtype=np.int64)
    src_all = np.concatenate([src, loops])
    dst_all = np.concatenate([dst, loops])
    deg = np.bincount(dst_all, minlength=NP_ALL)  # includes self-loop

    tile_of, slot_of, max_load = _balance_tiles(deg)
    ku = (max_load + P - 1) // P
    gid = tile_of.astype(np.int64) * P + slot_of  # node -> permuted row

    m_src = gid[src_all]
    m_tile = tile_of[dst_all].astype(np.int64)
    m_slot = slot_of[dst_all].astype(np.int64)

    # column 0 of every tile holds the 128 self-loop messages slot-aligned;
    # edge messages fill columns 1..ku-1
    is_loop_m = np.concatenate(
        [np.zeros(len(src), bool), np.ones(NP_ALL, bool)]
    )
    order = np.lexsort((~is_loop_m, m_tile))
    m_src, m_tile, m_slot, is_loop_m = (
        m_src[order],
        m_tile[order],
        m_slot[order],
        is_loop_m[order],
    )
    tile_counts = np.bincount(m_tile, minlength=T_ALL)
    tile_starts = np.concatenate([[0], np.cumsum(tile_counts)[:-1]])
    pos = np.arange(len(m_src)) - tile_starts[m_tile]
    mp = np.where(is_loop_m, m_slot, pos % P)
    mc = np.where(is_loop_m, 0, pos // P)

    cols = TILES_PER_CORE * ku
    midx = np.zeros((NCORES, P, cols), np.int32)
    mloc = np.full((NCORES, P, cols), -1.0, np.float32)
    core = m_tile // TILES_PER_CORE
    tl = m_tile % TILES_PER_CORE
    col = tl * ku + mc
    midx[core, mp, col] = m_src
    mloc[core, mp, col] = m_slot

    deg_inv = (1.0 / np.maximum(deg, 1.0)).astype(np.float32)
    deginv_core = deg_inv[np.argsort(gid)].reshape(NCORES, TILES_PER_CORE, P)
    deginv_core = np.ascontiguousarray(np.transpose(deginv_core, (0, 2, 1)))

    xp = np.zeros((NP_ALL, FIN), np.float32)
    xp[gid[:N]] = np.asarray(x, np.float32)
    return dict(
        ku=int(ku),
        midx=midx,
        mloc=mloc,
        deginv=deginv_core,
        xp=xp,
        gid=gid,
    )


# ----------------------------------------------------------------------------
# device program
# ----------------------------------------------------------------------------
def _padP(a):
    out = np.zeros((P, a.shape[1]), a.dtype)
    out[: a.shape[0]] = a
    return out


def _hilo(v):
    hi = _bf16(np.asarray(v, np.float32))
    lo = _bf16(np.asarray(v, np.float32) - np.asarray(hi, np.float32))
    return hi, lo


def _build_program(ku):
    phases = int(os.environ.get("KERNEL_PHASES", "3"))
    nc = bacc.Bacc()
    cols = TILES_PER_CORE * ku

    CF = 492
    CB = ku * P + cols
    CI = cols
    xe_in = nc.declare_dram_parameter(
        "xe", [TILES_PER_CORE, P, ku * FIN], BF16, isOutput=False
    )
    xlocT = nc.declare_dram_parameter("xlocT", [FIN, NPC], F32, isOutput=False)
    cf_in = nc.declare_dram_parameter("constf", [P, CF], F32, isOutput=False)
    cb_in = nc.declare_dram_parameter("constb", [P, CB], BF16, isOutput=False)
    ci_in = nc.declare_dram_parameter("consti", [P, CI], I32, isOutput=False)
    outloc = nc.declare_dram_parameter("outloc", [NPC, FOUT], F32, isOutput=True)

    z1loc = nc.dram_tensor("z1loc", [NPC, FW1], BF16)
    z1tab = nc.dram_tensor("z1tab", [NP_ALL, FW1], BF16, addr_space="Shared")
    z2loc = nc.dram_tensor("z2loc", [NPC, FW2], BF16)
    z2tab = nc.dram_tensor("z2tab", [NP_ALL, FW2], BF16, addr_space="Shared")
    sd1 = nc.dram_tensor("sd1", [NPC, 1], F32)
    sd2 = nc.dram_tensor("sd2", [NPC, 1], F32)

    groups = [list(range(NCORES))]

    with tile.TileContext(nc) as tc:
        with (
            tc.tile_pool(name="const", bufs=1) as cpool,
            tc.tile_pool(name="sbuf", bufs=4) as pool,
            tc.tile_pool(name="gath", bufs=14) as gpool,
            tc.tile_pool(name="psum", bufs=2, space="PSUM") as pacc,
            tc.tile_pool(name="psum1", bufs=1, space="PSUM") as ptp,
        ):
            def cload(ap, shape, dt, tag):
                t = cpool.tile(shape, dt, tag=tag)
                nc.sync.dma_start(out=t[:], in_=ap)
                return t

            cf = cload(cf_in[:, :], [P, CF], F32, tag="cf")
            cb = cload(cb_in[:, :], [P, CB], BF16, tag="cb")
            ci = cload(ci_in[:, :], [P, CI], I32, tag="ci")
            ident_t = cf[:, 0:128]
            dinv_t = cf[:, 128:226]
            b2r_t = cf[:, 226:258]
            bout_t = cf[:HID, 258:259]
            b1c_t = cf[:HID, 259:260]
            a1_t = cf[:HID, 260:264]
            a2_t = cf[:FOUT, 264:268]
            wout_t = cf[:FIN, 268:332]
            wroot_t = cf[:FIN, 332:396]
            w1_t = cf[:HID, 396:460]
            w2_t = cf[:HID, 460:492]
            iotak_t = cb[:, 0 : ku * P]
            mloc_t = cb[:, ku * P :]
            midx_t = ci[:, :]

            sdcol = cpool.tile([P, TILES_PER_CORE], F32, tag="sdcol")

            def sel_build(ti):
                """0/1 bf16 selection [P, ku, P] for tile ti."""
                sel = pool.tile([P, ku, P], BF16, tag="sel")
                nc.vector.tensor_tensor(
                    out=sel[:, :, :],
                    in0=mloc_t[:, ti * ku : (ti + 1) * ku, None].to_broadcast(
                        [P, ku, P]
                    ),
                    in1=iotak_t[:].rearrange("p (k c) -> p k c", k=ku),
                    op=OP.is_equal,
                )
                return sel

            def pack_from_T(hT_sb, w_t, a_t, fo, fw, zloc, ti):
                """Feature-major f32 activations hT_sb [fi, P] for tile ti ->
                z = h @ W, s_src / s_dst = z @ a, packed z-row to zloc,
                s_dst column stashed in sdcol."""
                zT_ps = ptp.tile([fo, P], F32, tag="zT")
                nc.tensor.matmul(
                    out=zT_ps[:], lhsT=w_t, rhs=hT_sb, start=True, stop=True
                )
                zT_sb = pool.tile([fo, P], F32, tag="zTsb")
                nc.scalar.copy(out=zT_sb[:], in_=zT_ps[:])
                sc_ps = ptp.tile([P, 4], F32, tag="sc")
                nc.tensor.matmul(
                    out=sc_ps[:], lhsT=zT_sb[:, :], rhs=a_t, start=True, stop=True
                )
                sc_sb = pool.tile([P, 4], F32, tag="sc_sb")
                nc.scalar.copy(out=sc_sb[:], in_=sc_ps[:, :])
                ssrc = pool.tile([P, 1], F32, tag="ssrc")
                nc.vector.tensor_tensor(
                    out=ssrc[:], in0=sc_sb[:, 0:1], in1=sc_sb[:, 1:2], op=OP.add
                )
                nc.vector.tensor_tensor(
                    out=sdcol[:, ti : ti + 1],
                    in0=sc_sb[:, 2:3],
                    in1=sc_sb[:, 3:4],
                    op=OP.add,
                )
                zr_ps = ptp.tile([P, fo], F32, tag="zr")
                nc.tensor.transpose(
                    out=zr_ps[:], in_=zT_sb[:, :], identity=ident_t[:fo, 0:fo]
                )
                zrow = pool.tile([P, fw], BF16, tag="zrow")
                nc.scalar.copy(out=zrow[:, 0:fo], in_=zr_ps[:, :])
                nc.vector.memset(zrow[:, fo : fo + 1], 1.0)
                nc.vector.memset(zrow[:, fo + 3 : fw], 0.0)
                nc.scalar.copy(out=zrow[:, fo + 1 : fo + 2], in_=ssrc[:, :])
                shi_f = pool.tile([P, 1], F32, tag="shif")
                nc.scalar.copy(out=shi_f[:], in_=zrow[:, fo + 1 : fo + 2])
                nc.vector.tensor_tensor(
                    out=zrow[:, fo + 2 : fo + 3],
                    in0=ssrc[:, :],
                    in1=shi_f[:, :],
                    op=OP.subtract,
                )
                nc.sync.dma_start(
                    out=zloc[ti * P : (ti + 1) * P, :], in_=zrow[:, :]
                )

            # ================= Layer 1: ClusterGCN =================
            for ti in range(TILES_PER_CORE):
                    msg = gpool.tile([P, ku * FIN], BF16, tag="msg1")
                    nc.sync.dma_start(out=msg[:, :], in_=xe_in[ti, :, :])
                    sel = sel_build(ti)
                    acc = pacc.tile([P, FIN], F32, tag="acc")
                    for k in range(ku):
                        nc.tensor.matmul(
                            out=acc[:],
                            lhsT=sel[:, k, :],
                            rhs=msg[:, k * FIN : (k + 1) * FIN],
                            start=(k == 0),
                            stop=(k == ku - 1),
                        )
                    agg = pool.tile([P, FIN], F32, tag="agg")
                    nc.scalar.mul(agg[:], acc[:, :], dinv_t[:, ti : ti + 1])
                    xT_sb = pool.tile([FIN, P], F32, tag="xT")
                    nc.sync.dma_start(
                        out=xT_sb[:], in_=xlocT[:, ti * P : (ti + 1) * P]
                    )
                    aT_ps = ptp.tile([FIN, P], F32, tag="tp")
                    nc.tensor.transpose(out=aT_ps[:], in_=agg[:, :], identity=ident_t)
                    aT_sb = pool.tile([FIN, P], F32, tag="aT")
                    nc.scalar.copy(out=aT_sb[:], in_=aT_ps[:])
                    hT_ps = ptp.tile([HID, P], F32, tag="hTp")
                    nc.tensor.matmul(
                        out=hT_ps[:], lhsT=wout_t, rhs=aT_sb[:, :],
                        start=True, stop=False,
                    )
                    nc.tensor.matmul(
                        out=hT_ps[:], lhsT=wroot_t, rhs=xT_sb[:, :],
                        start=False, stop=True,
                    )
                    h1T_sb = pool.tile([HID, P], F32, tag="h1T")
                    nc.scalar.activation(
                        out=h1T_sb[:], in_=hT_ps[:], func=AF.Relu, bias=bout_t
                    )
                    pack_from_T(h1T_sb[:, :], w1_t, a1_t, HID, FW1, z1loc, ti)
            nc.sync.dma_start(
                out=sd1[:, :].rearrange("(p t) one -> p (t one)", p=P),
                in_=sdcol[:, :],
            )
            if phases >= 1:
                tc.strict_bb_all_engine_barrier()
                nc.gpsimd.collective_compute(
                    "AllGather",
                    OP.bypass,
                    replica_groups=groups,
                    ins=[z1loc[:, :]],
                    outs=[z1tab[:, :]],
                )
                tc.strict_bb_all_engine_barrier()

            # ================= Layers 2 & 3: GAT =================
            def gat_layer(ztab, zself, sdt, fw, fo, w_t, a_t, zloc_next, fw_next, sd_next, last):
                fz = fw - 4
                sdl = cpool.tile([P, TILES_PER_CORE], F32, tag=f"sdl{fw}")
                nc.sync.dma_start(
                    out=sdl[:],
                    in_=sdt[:, :].rearrange("(p t) one -> p (t one)", p=P),
                )
                for ti in range(TILES_PER_CORE):
                        msg = gpool.tile([P, ku, fw], BF16, tag="msg2")
                        # column 0 = slot-aligned self-loop rows: contiguous
                        nc.sync.dma_start(
                            out=msg[:, 0, :],
                            in_=zself[ti * P : (ti + 1) * P, :],
                        )
                        for k in range(1, ku):
                            nc.gpsimd.indirect_dma_start(
                                out=msg[:, k, :],
                                out_offset=None,
                                in_=ztab[:, :],
                                in_offset=bass.IndirectOffsetOnAxis(
                                    ap=midx_t[:, ti * ku + k : ti * ku + k + 1],
                                    axis=0,
                                ),
                            )
                        sel = sel_build(ti)
                        # s_dst row-replication: transpose of broadcast column
                        sdf_ps = ptp.tile([P, P], F32, tag="sdf")
                        nc.tensor.transpose(
                            out=sdf_ps[:],
                            in_=sdl[:, ti : ti + 1].to_broadcast([P, P]),
                            identity=ident_t,
                        )
                        sdf_sb = pool.tile([P, P], BF16, tag="sdfb")
                        nc.scalar.copy(out=sdf_sb[:], in_=sdf_ps[:])
                        # sde2[p,k] = sum_c sel[p,k,c] * s_dst[c]
                        sds = pool.tile([P, ku, P], BF16, tag="sds")
                        nc.vector.tensor_tensor(
                            out=sds[:, :, :],
                            in0=sel[:, :, :],
                            in1=sdf_sb[:, None, :].to_broadcast([P, ku, P]),
                            op=OP.mult,
                        )
                        sde2 = pool.tile([P, ku], F32, tag="sde2")
                        nc.vector.tensor_reduce(
                            out=sde2[:, :],
                            in_=sds[:, :, :],
                            axis=mybir.AxisListType.X,
                            op=OP.add,
                        )
                        # logits l = s_src(hi+lo) + s_dst
                        l = pool.tile([P, ku], F32, tag="l")
                        nc.vector.tensor_tensor(
                            out=l[:],
                            in0=msg[:, :, fz + 1],
                            in1=msg[:, :, fz + 2],
                            op=OP.add,
                        )
                        nc.vector.tensor_tensor(
                            out=l[:], in0=l[:], in1=sde2[:, :], op=OP.add
                        )
                        # w = exp(leaky_relu(l)) = max(exp(l), exp(0.2*l))
                        w1e = pool.tile([P, ku], F32, tag="w1")
                        nc.scalar.activation(out=w1e[:], in_=l[:], func=AF.Exp)
                        w2e = pool.tile([P, ku], F32, tag="w2")
                        nc.scalar.activation(
                            out=w2e[:], in_=l[:], func=AF.Exp, scale=float(NEG)
                        )
                        wb = pool.tile([P, ku], BF16, tag="wb")
                        nc.vector.tensor_tensor(
                            out=wb[:], in0=w1e[:], in1=w2e[:], op=OP.max
                        )
                        # weighted messages (+denominator column fz)
                        mp = pool.tile([P, ku, fz + 1], BF16, tag="mp")
                        nc.vector.tensor_tensor(
                            out=mp[:, :, :],
                            in0=msg[:, :, 0 : fz + 1],
                            in1=wb[:, :, None].to_broadcast([P, ku, fz + 1]),
                            op=OP.mult,
                        )
                        acc = pacc.tile([P, fz + 1], F32, tag="acc")
                        for k in range(ku):
                            nc.tensor.matmul(
                                out=acc[:],
                                lhsT=sel[:, k, :],
                                rhs=mp[:, k, :],
                                start=(k == 0),
                                stop=(k == ku - 1),
                            )
                        den = pool.tile([P, 1], F32, tag="den")
                        nc.vector.tensor_scalar(
                            out=den[:], in0=acc[:, fz : fz + 1], scalar1=1e-30,
                            scalar2=None, op0=OP.max,
                        )
                        rec = pool.tile([P, 1], F32, tag="rec")
                        nc.vector.reciprocal(out=rec[:], in_=den[:])
                        if last:
                            h = pool.tile([P, fz], F32, tag="h")
                            nc.scalar.mul(h[:], acc[:, 0:fz], rec[:, :])
                            nc.vector.tensor_tensor(
                                out=h[:], in0=h[:], in1=b2r_t, op=OP.add
                            )
                            nc.sync.dma_start(
                                out=outloc[ti * P : (ti + 1) * P, :], in_=h[:, :]
                            )
                        else:
                            hsc = pool.tile([P, fz], F32, tag="hsc")
                            nc.scalar.mul(hsc[:], acc[:, 0:fz], rec[:, :])
                            hT_ps = ptp.tile([fz, P], F32, tag="tp")
                            nc.tensor.transpose(
                                out=hT_ps[:], in_=hsc[:, :], identity=ident_t
                            )
                            hT_sb = pool.tile([fz, P], F32, tag="h1T")
                            nc.scalar.activation(
                                out=hT_sb[:], in_=hT_ps[:], func=AF.Relu,
                                bias=b1c_t,
                            )
                            pack_from_T(
                                hT_sb[:, :], w_t, a_t, fw_next - 4, fw_next,
                                zloc_next, ti,
                            )
                if not last:
                    nc.sync.dma_start(
                        out=sd_next[:, :].rearrange("(p t) one -> p (t one)", p=P),
                        in_=sdcol[:, :],
                    )

            if phases >= 2:
                gat_layer(z1tab, z1loc, sd1, FW1, HID, w2_t, a2_t, z2loc, FW2, sd2, False)
            if phases >= 3:
                tc.strict_bb_all_engine_barrier()
                nc.gpsimd.collective_compute(
                    "AllGather",
                    OP.bypass,
                    replica_groups=groups,
                    ins=[z2loc[:, :]],
                    outs=[z2tab[:, :]],
                )
                tc.strict_bb_all_engine_barrier()
                gat_layer(z2tab, z2loc, sd2, FW2, FOUT, None, None, None, None, None, True)
            if phases < 3:
                for ti in range(TILES_PER_CORE):
                    zt = pool.tile([P, FOUT], F32, tag="h")
                    nc.vector.memset(zt[:], 0.0)
                    nc.sync.dma_start(out=outloc[ti * P : (ti + 1) * P, :], in_=zt[:, :])

    nc.finalize()
    return nc


# ----------------------------------------------------------------------------
# entry point
# ----------------------------------------------------------------------------
def kernel(
    x,
    edge_index,
    W_out,
    b_out,
    W_root,
    W1,
    a_src1,
    a_dst1,
    b1,
    W2,
    a_src2,
    a_dst2,
    b2,
    training=0,
    **_unused,
):
    pre = _preprocess(x, edge_index)
    ku = pre["ku"]
    _key = (ku, os.environ.get("KERNEL_PHASES", "3"))
    if _key not in _cache:
        _cache[_key] = _build_program(ku)
    nc = _cache[_key]

    iota = np.tile(np.arange(P, dtype=np.float32), (P, 1))
    iotak = np.tile(iota, (1, ku))
    ident = np.eye(P, dtype=np.float32)

    a1hi, a1lo = _hilo(np.asarray(a_src1, np.float32))
    a1dhi, a1dlo = _hilo(np.asarray(a_dst1, np.float32))
    a2hi, a2lo = _hilo(np.asarray(a_src2, np.float32))
    a2dhi, a2dlo = _hilo(np.asarray(a_dst2, np.float32))
    a1 = np.stack(
        [np.float32(a1hi), np.float32(a1lo), np.float32(a1dhi), np.float32(a1dlo)], 1
    )
    a2 = np.stack(
        [np.float32(a2hi), np.float32(a2lo), np.float32(a2dhi), np.float32(a2dlo)], 1
    )

    xp = pre["xp"]
    xp_b = _bf16(xp)
    in_maps = []
    for c in range(NCORES):
        mi = pre["midx"][c].reshape(P, TILES_PER_CORE, ku).astype(np.int64)
        xe = np.ascontiguousarray(
            np.transpose(xp_b[mi], (1, 0, 2, 3))
        ).reshape(TILES_PER_CORE, P, ku * FIN)
        in_maps.append(
            {
                "xe": xe,
                "xlocT": np.ascontiguousarray(xp[c * NPC : (c + 1) * NPC].T),
                "constf": np.concatenate(
                    [
                        ident,
                        pre["deginv"][c],
                        np.tile(np.asarray(b2, np.float32), (P, 1)),
                        _padP(np.asarray(b_out, np.float32).reshape(HID, 1)),
                        _padP(np.asarray(b1, np.float32).reshape(HID, 1)),
                        _padP(a1),
                        _padP(a2),
                        _padP(np.asarray(W_out, np.float32)),
                        _padP(np.asarray(W_root, np.float32)),
                        _padP(np.asarray(W1, np.float32)),
                        _padP(np.asarray(W2, np.float32)),
                    ],
                    axis=1,
                ),
                "constb": np.concatenate(
                    [_bf16(iotak), _bf16(pre["mloc"][c])], axis=1
                ),
                "consti": pre["midx"][c],
            }
        )

    trace = bool(os.environ.get("BASS_TRACE"))
    res = run_bass_kernel_spmd(
        nc, in_maps, list(range(NCORES)), trace=trace
    )
    global last_result
    last_result = res
    out_p = np.concatenate([res.results[c]["outloc"] for c in range(NCORES)], 0)
    out = out_p[pre["gid"][:N]]
    return np.asarray(out, np.float32)
